# revision 30
# baseline (speedup 1.0000x reference)
"""Causal multi-head attention (B=4, N=2048, D=1024, H=16, dk=dv=64) on 8 Trainium2
NeuronCores.

Sharding: tensor-parallel over heads — core c computes QKV projections and
attention for heads 2c, 2c+1. v2 restructures the baseline into a
batch-pipelined schedule: per batch b we emit QKV(b) -> attention(b) ->
staged per-batch AllToAll(b), with output projection of batch b-1 emitted in
the middle of attention(b). Tile's list scheduler then interleaves QKV
matmuls of the next batch into attention's exp-wait gaps, keeping the PE
dense (HAM stays at full clock) and hiding the collectives.

Attention uses the S^T layout (keys on partitions). Per (q-tile, key-tile)
iteration both heads' scores go into adjacent PSUM banks so ONE activation
instruction computes exp for both heads; diagonal band tiles use trapezoid
free dims (only valid queries) plus a single [128,128] triangle mask. P and
V are bf16 (fp32 PSUM accumulation); softmax denominators come from a ones
column appended to V. Normalization is deferred to after P@V.
"""

import numpy as np

B, N, D = 4, 2048, 1024
NCORES = 8
KT = D // 128            # 8 contraction tiles of d_model
TPB = N                  # 2048 tokens per batch
NT_B = TPB // 512        # 4 projection supertiles per batch
QT_B = TPB // 512        # 4 query tiles per batch
EV = TPB // NCORES       # 256 tokens per core per A2A event

_CACHE = {}
TRACE = False
LAST_EXEC_NS = None
LAST_RESULTS = None


def _build():
    import concourse.tile as tile
    from concourse import bacc, mybir

    F32 = mybir.dt.float32
    F32R = mybir.dt.float32r
    BF16 = mybir.dt.bfloat16
    Exp = mybir.ActivationFunctionType.Exp
    mult = mybir.AluOpType.mult

    nc = bacc.Bacc("TRN2", target_bir_lowering=False, debug=False, num_devices=NCORES)

    xT_d = nc.dram_tensor("xT", [D, B * N], F32, kind="ExternalInput")
    wq_d = nc.dram_tensor("wq", [D, 128], F32, kind="ExternalInput")
    wk_d = nc.dram_tensor("wk", [D, 128], F32, kind="ExternalInput")
    wv_d = nc.dram_tensor("wv", [D, 128], F32, kind="ExternalInput")
    bq_d = nc.dram_tensor("bq", [128, 1], F32, kind="ExternalInput")
    bk_d = nc.dram_tensor("bk", [128, 1], F32, kind="ExternalInput")
    bv_d = nc.dram_tensor("bv", [128, 1], F32, kind="ExternalInput")
    wo_d = nc.dram_tensor("wo", [D, D], F32, kind="ExternalInput")
    ident_d = nc.dram_tensor("ident", [128, 128], F32, kind="ExternalInput")
    tri_d = nc.dram_tensor("tri", [128, 128], F32, kind="ExternalInput")
    out_d = nc.dram_tensor("out", [B * EV, D], F32, kind="ExternalOutput")

    with tile.TileContext(nc) as tc:
        with (
            tc.tile_pool(name="dram", bufs=1, space="DRAM") as dram,
            tc.tile_pool(name="wts", bufs=1) as wts,
            tc.tile_pool(name="xp", bufs=4) as xp,
            tc.tile_pool(name="qkvp", bufs=2) as qkvp,
            tc.tile_pool(name="vtp", bufs=2) as vtp,
            tc.tile_pool(name="pbp", bufs=4) as pbp,
            tc.tile_pool(name="othp", bufs=2) as othp,
            tc.tile_pool(name="nrmp", bufs=2) as nrmp,
            tc.tile_pool(name="otp", bufs=2) as otp,
            tc.tile_pool(name="osbp", bufs=2) as osbp,
            # PSUM: scores 2x[128,1024] = 4 banks, PV accums = 2 banks,
            # misc (QKV/V-transpose/outproj) 2x[128,512] = 2 banks -> 8 total
            tc.tile_pool(name="scps", bufs=2, space="PSUM") as scps,
            tc.tile_pool(name="pvps", bufs=1, space="PSUM") as pvps,
            tc.tile_pool(name="mps", bufs=2, space="PSUM") as mps,
        ):
            # ---------------- prologue: constants and weights ----------------
            wq_s = wts.tile([128, KT * 128], F32R, name="wq_s")
            wk_s = wts.tile([128, KT * 128], F32R, name="wk_s")
            wv_s = wts.tile([128, KT * 128], F32R, name="wv_s")
            bq_s = wts.tile([128, 1], F32, name="bq_s")
            bk_s = wts.tile([128, 1], F32, name="bk_s")
            bv_s = wts.tile([128, 1], F32, name="bv_s")
            ident_f = wts.tile([128, 128], F32, name="ident_f")
            tri_f = wts.tile([128, 128], F32, name="tri_f")
            ident_b = wts.tile([128, 128], BF16, name="ident_b")
            tri_b = wts.tile([128, 128], BF16, name="tri_b")
            wo_s = wts.tile([128, KT * D], BF16, name="wo_s")

            nc.sync.dma_start(bq_s[:], bq_d[:])
            nc.sync.dma_start(bk_s[:], bk_d[:])
            nc.sync.dma_start(bv_s[:], bv_d[:])
            nc.sync.dma_start(ident_f[:], ident_d[:])
            nc.sync.dma_start(tri_f[:], tri_d[:])
            nc.vector.tensor_copy(ident_b[:], ident_f[:])
            nc.vector.tensor_copy(tri_b[:], tri_f[:])
            # row 0 of tri is all ones; rounded copy gives an f32r ones row
            ones64 = wts.tile([1, 64], F32R, name="ones64")
            nc.vector.tensor_copy(ones64[:], tri_f[0:1, 0:64])
            for w_s, w_d, eng in (
                (wq_s, wq_d, nc.sync),
                (wk_s, wk_d, nc.gpsimd),
                (wv_s, wv_d, nc.sync),
            ):
                for kk in range(KT):
                    eng.dma_start(
                        w_s[:, 128 * kk:128 * (kk + 1)],
                        w_d[128 * kk:128 * (kk + 1), :].bitcast(F32R),
                    )

            ot_dram = [
                [
                    dram.tile([NCORES, 128, 128], BF16, name=f"otd{b}_{h}")
                    for h in range(2)
                ]
                for b in range(B)
            ]
            a2a = [
                [
                    dram.tile([NCORES, 128, 128], BF16, name=f"a2a{b}_{h}")
                    for h in range(2)
                ]
                for b in range(B)
            ]

            def load_wo():
                # Wo^T, feat-major chunks; gpsimd (casting f32 -> bf16),
                # emitted after batch-0's x tiles so it never delays QKV(0).
                for kk in range(KT):
                    nc.gpsimd.dma_start(
                        wo_s[:, D * kk:D * (kk + 1)],
                        wo_d[128 * kk:128 * (kk + 1), :],
                    )

            def prefetch_x(b):
                # batched loads of the whole batch's x supertiles; emitted at
                # the START of the previous batch's attention so they precede
                # every collective trigger on the gpsimd queue and QKV(b) can
                # fill attention's exp-wait gaps across the full batch
                sups = []
                for tt in range(NT_B):
                    xsup = xp.tile([128, KT * 512], F32R, name="xsup")
                    csl = slice(N * b + 512 * tt, N * b + 512 * (tt + 1))
                    for h in range(2):
                        nc.gpsimd.dma_start(
                            xsup[:, 2048 * h:2048 * (h + 1)]
                            .rearrange("p (j c) -> p j c", c=512),
                            xT_d[512 * h:512 * (h + 1), csl]
                            .rearrange("(j p) c -> p j c", p=128),
                        )
                    sups.append(xsup)
                return sups

            def qkv_batch(b, xsups, pending_norm=None):
                qt = qkvp.tile([128, TPB], F32R, name="qt_b")
                kt = qkvp.tile([128, TPB], F32R, name="kt_b")
                vsb = qkvp.tile([128, 16 * 130], BF16, name="vsb_b")
                # ones columns for the softmax denominator, free index 65j+64
                nc.vector.memset(
                    vsb[:].rearrange("p (j c) -> p j c", c=65)[:, :, 64:65], 1.0
                )
                for tt in range(NT_B):
                    xsup = xsups[tt]
                    xts = [
                        xsup[:, 512 * kk:512 * (kk + 1)] for kk in range(KT)
                    ]
                    sl = slice(512 * tt, 512 * (tt + 1))
                    # q/k/v sequentially so only one PSUM accumulator is live
                    q_ps = mps.tile([128, 512], F32, name="q_ps", tag="mx")
                    for kk in range(KT):
                        nc.tensor.matmul(
                            q_ps[:], wq_s[:, 128 * kk:128 * (kk + 1)], xts[kk],
                            start=kk == 0, stop=kk == KT - 1,
                        )
                    if tt == 0 and pending_norm is not None:
                        # previous batch's final q-tile normalization: its DVE
                        # chain runs under the Q matmuls (emitted before the
                        # bias-add so the DVE reaches it immediately)
                        pending_norm()
                        pending_norm = None
                    nc.vector.tensor_scalar_add(qt[:, sl], q_ps[:], bq_s[:])
                    k_ps = mps.tile([128, 512], F32, name="k_ps", tag="mx")
                    for kk in range(KT):
                        nc.tensor.matmul(
                            k_ps[:], wk_s[:, 128 * kk:128 * (kk + 1)], xts[kk],
                            start=kk == 0, stop=kk == KT - 1,
                        )
                    nc.vector.tensor_scalar_add(kt[:, sl], k_ps[:], bk_s[:])
                    v_ps = mps.tile([128, 512], F32, name="v_ps", tag="mx")
                    for kk in range(KT):
                        nc.tensor.matmul(
                            v_ps[:], wv_s[:, 128 * kk:128 * (kk + 1)], xts[kk],
                            start=kk == 0, stop=kk == KT - 1,
                        )
                    vt_sb = vtp.tile([128, 512], BF16, name="vt_sb")
                    nc.vector.tensor_scalar_add(vt_sb[:], v_ps[:], bv_s[:])
                    # transpose V to key-major and interleave [Vh0 | 1 | Vh1 | 1]
                    for j in range(4):
                        vtr = mps.tile([128, 128], BF16, name="vtr", tag="mx")
                        nc.tensor.transpose(
                            vtr[:], vt_sb[:, 128 * j:128 * (j + 1)], ident_b[:]
                        )
                        base = (4 * tt + j) * 130
                        nc.vector.tensor_copy(
                            vsb[:, base:base + 130]
                            .rearrange("p (h c) -> p h c", h=2)[:, :, 0:64],
                            vtr[:].rearrange("p (h c) -> p h c", h=2),
                        )
                return qt, kt, vsb

            def outproj_ev(b, h):
                ot_t = otp.tile([128, NCORES * 128], BF16, name="ot_t")
                nc.sync.dma_start(
                    ot_t[:].rearrange("p (s c) -> p s c", c=128),
                    a2a[b][h][:].rearrange("s p c -> p s c"),
                )
                osb = osbp.tile([128, D], F32, name="osb")
                for half in range(2):
                    ps = mps.tile([128, 512], F32, name="op_ps", tag="mx")
                    for s in range(NCORES):
                        lhs = ot_t[:, 128 * s:128 * (s + 1)]
                        nc.tensor.matmul(
                            ps[:], lhs, wo_s[:, D * s + 512 * half:D * s + 512 * (half + 1)],
                            start=s == 0, stop=s == NCORES - 1,
                        )
                    nc.vector.tensor_copy(osb[:, 512 * half:512 * (half + 1)], ps[:])
                nc.sync.dma_start(
                    out_d[EV * b + 128 * h:EV * b + 128 * (h + 1), :], osb[:]
                )

            def attention_batch(b, qt, kt, vsb, oth0, oth1, hooks=None):
                def do_norm(o_ps0, o_ps1, qq):
                    # normalize by the ones-column row sums (both heads fused):
                    # D rows -> SBUF [2,512] -> fast reciprocal -> f32r round
                    # -> K=1 PE matmul broadcast -> SBUF -> multiply; stage
                    osl = slice(512 * qq, 512 * (qq + 1))
                    dsb = nrmp.tile([1, 1024], F32, name="dsb")
                    nc.vector.tensor_copy(dsb[:, 0:512], o_ps0[64:65, :])
                    nc.vector.tensor_copy(dsb[:, 512:1024], o_ps1[64:65, :])
                    rc = nrmp.tile([1, 1024], F32, name="rc")
                    nc.vector.reciprocal_approx_fast(rc[:], dsb[:])
                    rcr = nrmp.tile([1, 1024], F32R, name="rcr")
                    nc.vector.tensor_copy(rcr[:], rc[:])
                    for o_ps, oth, hsl in (
                        (o_ps0, oth0, slice(0, 512)),
                        (o_ps1, oth1, slice(512, 1024)),
                    ):
                        bc_ps = mps.tile([64, 512], F32, name="bc_ps", tag="mx")
                        nc.tensor.matmul(
                            bc_ps[:], ones64[:], rcr[:, hsl], start=True, stop=True
                        )
                        bc = nrmp.tile([64, 512], F32, name="bc", padded_shape=[128, 512])
                        nc.vector.tensor_copy(bc[:], bc_ps[:])
                        nc.vector.tensor_tensor(
                            oth[:, osl], o_ps[0:64, :], bc[:], op=mult
                        )
                    hh, lq = qq // 2, qq % 2
                    otd = ot_dram[b][hh][:].rearrange("i p c -> p i c")
                    nc.sync.dma_start(
                        otd[0:64, 4 * lq:4 * lq + 4],
                        oth0[:, osl].rearrange("p (i c) -> p i c", c=128),
                    )
                    nc.sync.dma_start(
                        otd[64:128, 4 * lq:4 * lq + 4],
                        oth1[:, osl].rearrange("p (i c) -> p i c", c=128),
                    )

                prev = None
                for qq in range(QT_B):
                    if hooks and hooks.get(qq):
                        hooks[qq]()
                    o_ps0 = pvps.tile([65, 512], F32, name="o_ps0")
                    o_ps1 = pvps.tile([65, 512], F32, name="o_ps1")
                    kmax = 4 * qq + 3

                    def pv(p_both, pkk, cols, qoff, last):
                        vb = 130 * pkk
                        f = pkk == 0
                        nc.tensor.matmul(
                            o_ps0[:, qoff:qoff + cols], vsb[:, vb:vb + 65],
                            p_both[:, 0:cols],
                            start=f, stop=last, skip_group_check=True,
                        )
                        nc.tensor.matmul(
                            o_ps1[:, qoff:qoff + cols], vsb[:, vb + 65:vb + 130],
                            p_both[:, 512:512 + cols],
                            start=f, stop=last, skip_group_check=True,
                        )

                    pend = None
                    for kk in range(kmax + 1):
                        if kk == 3 and prev is not None:
                            # deferred normalization of the previous q-tile:
                            # the DVE chain gets three iterations of slack, so
                            # the PE broadcast matmul won't stall the queue
                            do_norm(*prev)
                            prev = None
                            if qq == 2:
                                cc(b, 0)
                        r = kk - 4 * qq
                        cols = 512 if r < 0 else 512 - 128 * r
                        qoff = 0 if r < 0 else 128 * r
                        qsl = slice(512 * qq + qoff, 512 * (qq + 1))
                        ksl = slice(128 * kk, 128 * (kk + 1))
                        s_both = scps.tile([128, 1024], F32, name="s_both")
                        nc.tensor.matmul(
                            s_both[:, 0:cols], kt[0:64, ksl], qt[0:64, qsl],
                            start=True, stop=True,
                        )
                        nc.tensor.matmul(
                            s_both[:, 512:512 + cols], kt[64:128, ksl], qt[64:128, qsl],
                            start=True, stop=True, tile_position=(64, 0),
                        )
                        p_both = pbp.tile([128, 1024], BF16, name="p_both")
                        sv = s_both[:].rearrange("p (h c) -> p h c", c=512)[:, :, 0:cols]
                        pw = p_both[:].rearrange("p (h c) -> p h c", c=512)[:, :, 0:cols]
                        nc.scalar.activation(pw, sv, Exp, scale=0.125)
                        if r >= 0:
                            # first 128 columns of each head's region are the
                            # diagonal triangle
                            nc.vector.tensor_tensor(
                                p_both[:, 0:128], p_both[:, 0:128], tri_b[:], op=mult
                            )
                            nc.vector.tensor_tensor(
                                p_both[:, 512:640], p_both[:, 512:640], tri_b[:], op=mult
                            )
                        if pend is not None:
                            pv(*pend, last=False)
                        pend = (p_both, kk, cols, qoff)
                    pv(*pend, last=True)
                    prev = (o_ps0, o_ps1, qq)
                return lambda: do_norm(*prev)

            def cc(b, h):
                nc.gpsimd.collective_compute(
                    "AllToAll",
                    mybir.AluOpType.bypass,
                    replica_groups=[list(range(NCORES))],
                    ins=[ot_dram[b][h][:]],
                    outs=[a2a[b][h][:]],
                )

            def hook1(bb):
                outproj_ev(bb - 1, 0)
                cc(bb - 1, 1)

            pending_norm = None
            nxt_xs = {}
            xs = prefetch_x(0)
            for b in range(B):
                qt, kt, vsb = qkv_batch(b, xs, pending_norm)
                if b == 0:
                    load_wo()
                oth0 = othp.tile([64, TPB], BF16, name="oth0", padded_shape=[128, TPB])
                oth1 = othp.tile([64, TPB], BF16, name="oth1", padded_shape=[128, TPB])
                hooks = {}
                if b + 1 < B:
                    hooks[0] = lambda bb=b: nxt_xs.__setitem__("xs", prefetch_x(bb + 1))
                if b >= 1:
                    hooks[1] = lambda bb=b: hook1(bb)
                    hooks[3] = lambda bb=b: outproj_ev(bb - 1, 1)
                pending_norm = attention_batch(b, qt, kt, vsb, oth0, oth1, hooks=hooks)
                xs = nxt_xs.get("xs")
            pending_norm()
            outproj_ev(B - 1, 0)
            cc(B - 1, 1)
            outproj_ev(B - 1, 1)

    nc.compile()
    return nc


def _host_prep(inputs):
    x = np.asarray(inputs["x"], np.float32)
    Wq = np.asarray(inputs["Wq"], np.float32)
    bq = np.asarray(inputs["bq"], np.float32)
    Wk = np.asarray(inputs["Wk"], np.float32)
    bk = np.asarray(inputs["bk"], np.float32)
    Wv = np.asarray(inputs["Wv"], np.float32)
    bv = np.asarray(inputs["bv"], np.float32)
    Wo = np.asarray(inputs["Wo"], np.float32)

    xT = np.ascontiguousarray(x.reshape(B * N, D).T)
    woT = np.ascontiguousarray(Wo.T)
    ident = np.eye(128, dtype=np.float32)
    k_idx = np.arange(128)[:, None]
    j_idx = np.arange(128)[None, :]
    tri = (j_idx >= k_idx).astype(np.float32)

    in_maps = []
    for c in range(NCORES):
        sl = slice(128 * c, 128 * (c + 1))
        in_maps.append({
            "xT": xT,
            "wq": np.ascontiguousarray(Wq[sl].T),
            "wk": np.ascontiguousarray(Wk[sl].T),
            "wv": np.ascontiguousarray(Wv[sl].T),
            "bq": np.ascontiguousarray(bq[sl].reshape(128, 1)),
            "bk": np.ascontiguousarray(bk[sl].reshape(128, 1)),
            "bv": np.ascontiguousarray(bv[sl].reshape(128, 1)),
            "wo": woT,
            "ident": ident,
            "tri": tri,
        })
    return in_maps


def kernel(**inputs):
    global LAST_EXEC_NS, LAST_RESULTS
    from concourse.bass_utils import run_bass_kernel_spmd

    if "nc" not in _CACHE:
        _CACHE["nc"] = _build()
    nc = _CACHE["nc"]
    in_maps = _host_prep(inputs)
    res = run_bass_kernel_spmd(nc, in_maps, list(range(NCORES)), trace=TRACE)
    LAST_EXEC_NS = res.exec_time_ns
    LAST_RESULTS = res
    out = np.empty((B, N, D), np.float32)
    for c in range(NCORES):
        co = res.results[c]["out"].reshape(B, 2, 128, D)
        for h in range(2):
            out[:, 1024 * h + 128 * c:1024 * h + 128 * (c + 1), :] = co[:, h]
    return out
